# revision 5
# baseline (speedup 1.0000x reference)
"""MinGRU layer Trainium2 kernel.

Math (per batch b):
    g = x @ Wg + bg ; v = x @ Wv + bv ; d = x @ Wd + bd
    xs = sigmoid(g) * tanh(v) ; a = 0.001 + 0.998 * sigmoid(d)
    h_t = a_t * h_{t-1} + xs_t  (h_0 = 0, scan over time S)

Sharding: 8 cores = 4 batches x 2 halves of the 1024 output features.
Each core computes h^T[e, s] for its (b, e-half) with zero cross-core
communication; the time recurrence runs on-chip via the VectorE
TensorTensorScan instruction (time on the free axis, features on
partitions).

Host side feeds x transposed (d-major) so every device access is
contiguous; matmuls run in float32r (full-rate fp32 mode on the PE).
"""

import os
import sys

for _p in ("/opt/trn_rl_repo", "/root/.axon_site/_ro/trn_rl_repo"):
    if os.path.isdir(_p) and _p not in sys.path:
        sys.path.insert(0, _p)

import numpy as np

import concourse.bass as bass
import concourse.mybir as mybir
from concourse import bass_utils

B, S, D = 4, 4096, 1024
E = 512                # output features per core (D / 2)
NCH = 8                # time chunks
SC = S // NCH          # chunk length (512)
KT = D // 128          # contraction tiles (8)
JB = E // 128          # output-feature blocks per core (4)
NGRP = 3 * JB          # psum groups per chunk (12)

F32 = mybir.dt.float32
F32R = mybir.dt.float32r
AF = mybir.ActivationFunctionType
OP = mybir.AluOpType


def _build_bass():
    nc = bass.Bass("TRN2", target_bir_lowering=False, debug=False, num_devices=8)

    xt_d = nc.dram_tensor("xt", [D, S], F32R, kind="ExternalInput").ap()
    w_d = nc.dram_tensor("w", [3, D, E], F32R, kind="ExternalInput").ap()
    bias_d = nc.dram_tensor("bias", [128, 3 * JB], F32, kind="ExternalInput").ap()
    ht_d = nc.dram_tensor("ht", [E, S], F32, kind="ExternalOutput").ap()

    from contextlib import ExitStack

    with ExitStack() as ctx:
        block = ctx.enter_context(nc.Block())
        sem_ld = ctx.enter_context(nc.semaphore("sem_ld"))
        sem_pe = ctx.enter_context(nc.semaphore("sem_pe"))
        sem_act = ctx.enter_context(nc.semaphore("sem_act"))
        sem_dve = ctx.enter_context(nc.semaphore("sem_dve"))
        sem_st = ctx.enter_context(nc.semaphore("sem_st"))
        w_sb = ctx.enter_context(nc.sbuf_tensor("w_sb", [128, 3, KT, E], F32R))
        xt_sb = ctx.enter_context(nc.sbuf_tensor("xt_sb", [128, 2, KT, SC], F32R))
        bias_sb = ctx.enter_context(nc.sbuf_tensor("bias_sb", [128, 3 * JB], F32))
        sig_g = ctx.enter_context(nc.sbuf_tensor("sig_g", [128, 2, JB, SC], F32))
        tanh_v = ctx.enter_context(nc.sbuf_tensor("tanh_v", [128, 2, JB, SC], F32))
        sig_d = ctx.enter_context(nc.sbuf_tensor("sig_d", [128, 1, JB, SC], F32))
        a_t = ctx.enter_context(nc.sbuf_tensor("a_t", [128, 2, JB, SC], F32))
        xs_t = ctx.enter_context(nc.sbuf_tensor("xs_t", [128, 1, JB, SC], F32))
        h_t = ctx.enter_context(nc.sbuf_tensor("h_t", [128, 2, JB, SC], F32))
        psum = []
        for i in range(8):
            ps_i = ctx.enter_context(nc.psum_tensor(f"ps{i}", [128, SC], F32))
            psum.append(ps_i)

        # x^T viewed as [p, k, s]; row index of xt is d = 128*k + p
        xt_view = xt_d.rearrange("(k p) s -> p k s", p=128)
        # weights viewed as [p, proj, k, e]
        w_view = w_d.rearrange("q (k p) e -> p q k e", p=128)
        # h^T viewed as [p, j, s]; row index of ht is e = 128*j + p
        ht_view = ht_d.rearrange("(j p) s -> p j s", p=128)

        # ACT op index after op (c, j, i): 16c + 4j + i+1 (i in 0..3)
        def act_idx(c, j, i):
            return 16 * c + 4 * j + i + 1

        # PE group index: G = 12c + 3j + p
        def grp(c, j, p):
            return NGRP * c + 3 * j + p

        @block.gpsimd
        def _(gpsimd):
            gpsimd.dma_start(w_sb[:], w_view).then_inc(sem_ld, 16)
            gpsimd.dma_start(bias_sb[:], bias_d).then_inc(sem_ld, 16)
            for c in range(NCH):
                if c >= 2:
                    # xt slot c%2 was read by chunk c-2's matmuls
                    gpsimd.wait_ge(sem_pe, NGRP * (c - 1))
                gpsimd.dma_start(
                    xt_sb[:, c % 2, :, :], xt_view[:, :, SC * c : SC * (c + 1)]
                ).then_inc(sem_ld, 16)

        @block.tensor
        def _(tensor):
            tensor.wait_ge(sem_ld, 16)  # weights resident
            for c in range(NCH):
                tensor.wait_ge(sem_ld, 16 * c + 48)  # chunk c resident
                for j in range(JB):
                    for p in range(3):
                        g = grp(c, j, p)
                        if g >= 8:
                            # bank g%8 must have been drained by ACT
                            gp = g - 8
                            cp, rp = divmod(gp, NGRP)
                            jp, pp = divmod(rp, 3)
                            tensor.wait_ge(sem_act, act_idx(cp, jp, pp))
                        bank = psum[g % 8]
                        for k in range(KT):
                            mm = tensor.matmul(
                                bank[:],
                                w_sb[:, p, k, 128 * j : 128 * (j + 1)],
                                xt_sb[:, c % 2, k, :],
                                start=(k == 0),
                                stop=(k == KT - 1),
                            )
                        mm.then_inc(sem_pe, 1)

        @block.scalar
        def _(scalar):
            scalar.wait_ge(sem_ld, 32)  # biases resident
            for c in range(NCH):
                if c >= 2:
                    # sig_g/tanh_v/a slots (c%2) were read by chunk c-2's DVE ops
                    scalar.wait_ge(sem_dve, 8 * (c - 2) + 8)
                c2 = c % 2
                for j in range(JB):
                    scalar.wait_ge(sem_pe, grp(c, j, 0) + 1)
                    scalar.activation(
                        sig_g[:, c2, j, :],
                        psum[grp(c, j, 0) % 8][:],
                        AF.Sigmoid,
                        bias=bias_sb[:, 3 * j : 3 * j + 1],
                    ).then_inc(sem_act, 1)
                    scalar.wait_ge(sem_pe, grp(c, j, 1) + 1)
                    scalar.activation(
                        tanh_v[:, c2, j, :],
                        psum[grp(c, j, 1) % 8][:],
                        AF.Tanh,
                        bias=bias_sb[:, 3 * j + 1 : 3 * j + 2],
                    ).then_inc(sem_act, 1)
                    scalar.wait_ge(sem_pe, grp(c, j, 2) + 1)
                    scalar.activation(
                        sig_d[:, 0, j, :],
                        psum[grp(c, j, 2) % 8][:],
                        AF.Sigmoid,
                        bias=bias_sb[:, 3 * j + 2 : 3 * j + 3],
                    ).then_inc(sem_act, 1)
                    # a = 0.998*sigmoid(d) + 0.001 ; wait for own sigma_d
                    scalar.wait_ge(sem_act, act_idx(c, j, 2))
                    scalar.activation(
                        a_t[:, c2, j, :],
                        sig_d[:, 0, j, :],
                        AF.Copy,
                        bias=0.001,
                        scale=0.998,
                    ).then_inc(sem_act, 1)

        @block.vector
        def _(vector):
            for c in range(NCH):
                if c >= 2:
                    # h slot c%2 was read by store c-2
                    vector.wait_ge(sem_st, 16 * (c - 1))
                c2 = c % 2
                for j in range(JB):
                    vector.wait_ge(sem_act, act_idx(c, j, 1))
                    vector.tensor_tensor(
                        xs_t[:, 0, j, :],
                        sig_g[:, c2, j, :],
                        tanh_v[:, c2, j, :],
                        OP.mult,
                    ).then_inc(sem_dve, 1)
                    vector.wait_ge(sem_act, act_idx(c, j, 3))
                    init = 0.0 if c == 0 else h_t[:, (c - 1) % 2, j, SC - 1 : SC]
                    vector.tensor_tensor_scan(
                        h_t[:, c2, j, :],
                        a_t[:, c2, j, :],
                        xs_t[:, 0, j, :],
                        init,
                        OP.mult,
                        OP.add,
                    ).then_inc(sem_dve, 1)

        @block.sync
        def _(sync):
            for c in range(NCH):
                sync.wait_ge(sem_dve, 8 * c + 8)
                sync.dma_start(
                    ht_view[:, :, SC * c : SC * (c + 1)], h_t[:, c % 2, :, :]
                ).then_inc(sem_st, 16)

    return nc


_NC_CACHE = None


def _build_in_maps(inputs):
    x = np.asarray(inputs["x"], dtype=np.float32)
    Wg = np.asarray(inputs["Wg"], dtype=np.float32)
    bg = np.asarray(inputs["bg"], dtype=np.float32)
    Wv = np.asarray(inputs["Wv"], dtype=np.float32)
    bv = np.asarray(inputs["bv"], dtype=np.float32)
    Wd = np.asarray(inputs["Wd"], dtype=np.float32)
    bd = np.asarray(inputs["bd"], dtype=np.float32)

    in_maps = []
    for core in range(8):
        b, eh = divmod(core, 2)
        sl = slice(E * eh, E * (eh + 1))
        xt = np.ascontiguousarray(x[b].T)                       # (D, S)
        w = np.ascontiguousarray(
            np.stack([Wg[:, sl], Wv[:, sl], Wd[:, sl]], axis=0)
        )                                                       # (3, D, E)
        bias = np.empty((128, 3 * JB), dtype=np.float32)
        for pi, barr in enumerate((bg[sl], bv[sl], bd[sl])):
            b4 = barr.reshape(JB, 128)
            for j in range(JB):
                bias[:, 3 * j + pi] = b4[j]
        in_maps.append({"xt": xt, "w": w, "bias": bias})
    return in_maps


def kernel(**inputs: np.ndarray) -> np.ndarray:
    global _NC_CACHE
    if _NC_CACHE is None:
        _NC_CACHE = _build_bass()
    nc = _NC_CACHE

    in_maps = _build_in_maps(inputs)
    res = bass_utils.run_bass_kernel_spmd(nc, in_maps, core_ids=list(range(8)))

    out = np.empty((B, S, D), dtype=np.float32)
    for core in range(8):
        b, eh = divmod(core, 2)
        out[b, :, E * eh : E * (eh + 1)] = res.results[core]["ht"].T
    return out
